# revision 1
# baseline (speedup 1.0000x reference)
"""NoisyDense forward for Trainium2, 8-core tensor-parallel.

out = relu(x @ (w_mu + w_sigma * outer(eps_in, eps_out)) + b_mu + b_sigma*eps_out)

Sharding: 2-way over batch x 4-way over units (8 cores).
Per core: x_shard [2048, 4096] (batch rows), w shards [4096, 1024] (unit cols).
On-chip per core:
  - materialize noisy W shard once in SBUF, [128, 1024] fp32r k-tiles
  - stream x in 128-row panels, PE-transpose 128x128 tiles packed 4-per-PSUM
    bank (fp32 has no DMA transpose), fp32r matmuls (1 cyc/row @ N=512)
  - bias add + relu on DVE during PSUM eviction

Two kernel variants:
  - "rowsig": w_sigma rows are all identical (true for NoisyDense init:
    w_sigma = full(sigma)); only w_sigma[0, :] is shipped, saving a 16.8MB
    per-core stream. Selected at runtime after an exact host-side check.
  - "general": arbitrary w_sigma, full stream.

fp32r note: the BIR verifier requires every producer of an fp32r-matmul
operand to emit dtype float32r itself (engines round on write), so the
x / w_mu DRAM tensors and all tiles on the matmul path are float32r
end-to-end. numpy view is float32 either way.
"""

import numpy as np

BATCH = 4096
IN_DIM = 4096
UNITS = 4096
MSHARDS = 2
NSHARDS = 4
MS = BATCH // MSHARDS      # 2048 rows of x per core
NS = UNITS // NSHARDS      # 1024 units per core
P = 128
KO = IN_DIM // P           # 32 k-tiles
MP = MS // P               # 16 m-panels per core
NFREE = 512                # matmul moving free dim (one PSUM bank of fp32)
NT = NS // NFREE           # 2 n-tiles per core

_NC_CACHE = {}


def _build(variant="rowsig", mm_dtype_name="float32r"):
    from concourse import bacc
    import concourse.mybir as mybir
    import concourse.tile as tile
    from concourse.masks import make_identity

    f32 = mybir.dt.float32
    mdt = getattr(mybir.dt, mm_dtype_name)
    rowsig = variant == "rowsig"

    nc = bacc.Bacc(None, target_bir_lowering=False, dynamic_dma_scratch_size=2048)

    x_d = nc.dram_tensor("x_s", [MS, IN_DIM], mdt, kind="ExternalInput")
    wmu_d = nc.dram_tensor("wmu_s", [IN_DIM, NS], mdt, kind="ExternalInput")
    if rowsig:
        wsigr_d = nc.dram_tensor("wsig_row", [NS], f32, kind="ExternalInput")
    else:
        wsig_d = nc.dram_tensor("wsig_s", [IN_DIM, NS], f32, kind="ExternalInput")
    bmu_d = nc.dram_tensor("bmu_s", [NS], f32, kind="ExternalInput")
    bsig_d = nc.dram_tensor("bsig_s", [NS], f32, kind="ExternalInput")
    eout_d = nc.dram_tensor("eout_s", [NS], f32, kind="ExternalInput")
    ein_d = nc.dram_tensor("eps_in", [IN_DIM], f32, kind="ExternalInput")
    out_d = nc.dram_tensor("out_s", [MS, NS], f32, kind="ExternalOutput")

    mult = mybir.AluOpType.mult
    add = mybir.AluOpType.add

    TG = 4            # transposes packed per PSUM bank
    NTG = KO // TG    # 8 transpose groups per panel
    WSC = 2           # wsig staging chunk k-tiles (general variant)

    with tile.TileContext(nc) as tc:
        with (
            tc.tile_pool(name="const", bufs=1) as const,
            tc.tile_pool(name="wpool", bufs=1) as wpool,
            tc.tile_pool(name="wsig", bufs=2) as wsigp,
            tc.tile_pool(name="xnat", bufs=2 if rowsig else 1) as xnat,
            tc.tile_pool(name="xt", bufs=2) as xtp,
            tc.tile_pool(name="outp", bufs=1) as outp,
            tc.tile_pool(name="ps", bufs=6, space="PSUM") as psp,
            tc.tile_pool(name="pt", bufs=2, space="PSUM") as ptp,
        ):
            # ---- constants ----
            ident_f = const.tile([P, P], f32, tag="identf")
            make_identity(nc, ident_f)
            if mdt != f32:
                ident = const.tile([P, P], mdt, tag="ident")
                nc.vector.tensor_copy(out=ident[:], in_=ident_f[:])
            else:
                ident = ident_f

            eps_in_sb = const.tile([P, KO], f32, tag="epsin")
            with nc.allow_non_contiguous_dma(reason="one-time 16KB strided load"):
                nc.sync.dma_start(
                    eps_in_sb[:],
                    ein_d[:].bitcast(f32).rearrange("(ko ki) -> ki ko", ki=P),
                )

            # bias rows broadcast to all partitions straight from DRAM
            eout_b = const.tile([P, NS], f32, tag="eoutb")
            bsg_b = const.tile([P, NS], f32, tag="sgslot")
            b_b = const.tile([P, NS], f32, tag="bb")
            with nc.allow_non_contiguous_dma(reason="one-time row broadcasts"):
                nc.sync.dma_start(eout_b[:], eout_d[None, :].to_broadcast([P, NS]))
                nc.sync.dma_start(bsg_b[:], bsig_d[None, :].to_broadcast([P, NS]))
                nc.sync.dma_start(b_b[:], bmu_d[None, :].to_broadcast([P, NS]))
            # b = b_mu + b_sigma * eps_out
            nc.vector.tensor_mul(bsg_b[:], bsg_b[:], eout_b[:])
            nc.vector.tensor_add(b_b[:], b_b[:], bsg_b[:])

            if rowsig:
                # sigout_b[n] = w_sigma[0,n] * eps_out[n], bcast over partitions
                sigout_b = const.tile([P, NS], f32, tag="sgslot")
                with nc.allow_non_contiguous_dma(reason="one-time row broadcast"):
                    nc.sync.dma_start(
                        sigout_b[:], wsigr_d[None, :].to_broadcast([P, NS])
                    )
                nc.vector.tensor_mul(sigout_b[:], sigout_b[:], eout_b[:])

            # ---- x loads for the first two panels, before the bulky w DMAs,
            # so the PE has transpose work from t~0 ----
            def issue_x(pm):
                xa = xnat.tile([P, IN_DIM // 2], mdt, tag="xa")
                nc.sync.dma_start(xa[:], x_d[pm * P : (pm + 1) * P, 0 : IN_DIM // 2])
                xb = xnat.tile([P, IN_DIM // 2], mdt, tag="xb")
                nc.sync.dma_start(
                    xb[:], x_d[pm * P : (pm + 1) * P, IN_DIM // 2 : IN_DIM]
                )
                return xa, xb

            pre_x = {0: issue_x(0)}
            if rowsig:
                pre_x[1] = issue_x(1)

            # ---- w_mu load + noisy-W materialization, group by group ----
            wmu_r = wmu_d[:].rearrange("(ko ki) n -> ki ko n", ki=P)
            if not rowsig:
                wsig_r = wsig_d[:].rearrange("(ko ki) n -> ki ko n", ki=P)
            w_groups = []
            for g in range(KO // 8):
                wt = wpool.tile([P, 8, NS], mdt, tag=f"w{g}")
                nc.sync.dma_start(wt[:, 0:4, :], wmu_r[:, g * 8 : g * 8 + 4, :])
                nc.sync.dma_start(wt[:, 4:8, :], wmu_r[:, g * 8 + 4 : (g + 1) * 8, :])
                w_groups.append(wt)
                if rowsig:
                    for j in range(8):
                        ko = g * 8 + j
                        # w[ki,ko,:] = w_mu[ki,ko,:] + eps_in[ki,ko]*sigout_b
                        nc.vector.scalar_tensor_tensor(
                            out=wt[:, j, :],
                            in0=sigout_b[:],
                            scalar=eps_in_sb[:, ko : ko + 1],
                            in1=wt[:, j, :],
                            op0=mult,
                            op1=add,
                        )
                else:
                    for c in range(8 // WSC):
                        ws = wsigp.tile([P, WSC, NS], f32, tag="ws")
                        nc.sync.dma_start(
                            ws[:],
                            wsig_r[:, g * 8 + c * WSC : g * 8 + (c + 1) * WSC, :],
                        )
                        for j in range(WSC):
                            ko = g * 8 + c * WSC + j
                            nc.vector.scalar_tensor_tensor(
                                out=ws[:, j, :],
                                in0=eout_b[:],
                                scalar=eps_in_sb[:, ko : ko + 1],
                                in1=ws[:, j, :],
                                op0=mult,
                                op1=mult,
                            )
                            nc.vector.tensor_add(
                                wt[:, ko % 8, :], wt[:, ko % 8, :], ws[:, j, :]
                            )

            def w_slice(ko, nt):
                return w_groups[ko // 8][:, ko % 8, nt * NFREE : (nt + 1) * NFREE]

            # ---- panels: transpose x tiles on PE (packed 4/bank), matmuls ----
            def make_transpose_ops(pm):
                if pm in pre_x:
                    xa, xb = pre_x.pop(pm)
                else:
                    xa, xb = issue_x(pm)
                xts = [None] * NTG
                ops = []

                def mk(g):
                    def op():
                        pt = ptp.tile([P, TG * P], mdt, tag="pt")
                        for j in range(TG):
                            ko = g * TG + j
                            half = xa if ko < KO // 2 else xb
                            jj = ko % (KO // 2)
                            src = half[:, jj * P : (jj + 1) * P]
                            nc.tensor.matmul(
                                pt[:, j * P : (j + 1) * P],
                                src,
                                ident[:],
                                is_transpose=True,
                                start=(j == 0),
                                stop=(j == TG - 1),
                            )
                        t = xtp.tile([P, TG * P], mdt, tag=f"xt{g}")
                        if g % 2 == 0:
                            nc.vector.tensor_copy(out=t[:], in_=pt[:])
                        else:
                            nc.scalar.copy(out=t[:], in_=pt[:])
                        xts[g] = t

                    return op

                for g in range(NTG):
                    ops.append(mk(g))
                return ops, xts

            def lhsT(xts, ko):
                return xts[ko // TG][:, (ko % TG) * P : (ko % TG + 1) * P]

            prev_xts = None
            for mi in range(MP + 1):
                if mi < MP:
                    t_ops, cur_xts = make_transpose_ops(mi)
                else:
                    t_ops, cur_xts = [], None

                if prev_xts is None:
                    for op in t_ops:
                        op()
                else:
                    pm = mi - 1
                    ti = 0
                    ot = outp.tile([P, NS], f32, tag="ot")
                    for nt in range(NT):
                        ps = psp.tile([P, NFREE], f32, tag="ps")
                        for ko in range(KO):
                            nc.tensor.matmul(
                                ps[:],
                                lhsT(prev_xts, ko),
                                w_slice(ko, nt),
                                start=(ko == 0),
                                stop=(ko == KO - 1),
                            )
                            if ko % 8 == 7 and ti < len(t_ops):
                                t_ops[ti]()
                                ti += 1
                        nc.vector.tensor_add(
                            ot[:, nt * NFREE : (nt + 1) * NFREE],
                            ps[:],
                            b_b[:, nt * NFREE : (nt + 1) * NFREE],
                        )
                    nc.vector.tensor_scalar_max(ot[:], ot[:], 0.0)
                    nc.sync.dma_start(out_d[pm * P : (pm + 1) * P, :], ot[:])
                    while ti < len(t_ops):
                        t_ops[ti]()
                        ti += 1
                prev_xts = cur_xts

    nc.compile()
    return nc


def get_nc(variant="rowsig", mm_dtype_name="float32r"):
    key = (variant, mm_dtype_name)
    if key not in _NC_CACHE:
        _NC_CACHE[key] = _build(variant, mm_dtype_name)
    return _NC_CACHE[key]


def pick_variant(w_sigma):
    w_sigma = np.asarray(w_sigma)
    return "rowsig" if bool((w_sigma == w_sigma[0:1, :]).all()) else "general"


def shard_inputs(x, w_mu, w_sigma, b_mu, b_sigma, eps_in, eps_out, variant="rowsig"):
    x = np.asarray(x, dtype=np.float32)
    w_mu = np.asarray(w_mu, dtype=np.float32)
    w_sigma = np.asarray(w_sigma, dtype=np.float32)
    b_mu = np.asarray(b_mu, dtype=np.float32)
    b_sigma = np.asarray(b_sigma, dtype=np.float32)
    eps_in = np.asarray(eps_in, dtype=np.float32)
    eps_out = np.asarray(eps_out, dtype=np.float32)

    in_maps = []
    for c in range(MSHARDS * NSHARDS):
        mr, ncol = divmod(c, NSHARDS)
        msl = slice(mr * MS, (mr + 1) * MS)
        nsl = slice(ncol * NS, (ncol + 1) * NS)
        m = {
            "x_s": np.ascontiguousarray(x[msl, :]),
            "wmu_s": np.ascontiguousarray(w_mu[:, nsl]),
            "bmu_s": np.ascontiguousarray(b_mu[nsl]),
            "bsig_s": np.ascontiguousarray(b_sigma[nsl]),
            "eout_s": np.ascontiguousarray(eps_out[nsl]),
            "eps_in": eps_in,
        }
        if variant == "rowsig":
            m["wsig_row"] = np.ascontiguousarray(w_sigma[0, nsl])
        else:
            m["wsig_s"] = np.ascontiguousarray(w_sigma[:, nsl])
        in_maps.append(m)
    return in_maps


def unshard_output(results):
    out = np.empty((BATCH, UNITS), dtype=np.float32)
    for c, rmap in enumerate(results):
        mr, ncol = divmod(c, NSHARDS)
        out[mr * MS : (mr + 1) * MS, ncol * NS : (ncol + 1) * NS] = rmap["out_s"]
    return out


def kernel(x, w_mu, w_sigma, b_mu, b_sigma, eps_in, eps_out):
    from concourse.bass_utils import run_bass_kernel_spmd

    variant = pick_variant(w_sigma)
    nc = get_nc(variant)
    in_maps = shard_inputs(
        x, w_mu, w_sigma, b_mu, b_sigma, eps_in, eps_out, variant=variant
    )
    res = run_bass_kernel_spmd(nc, in_maps, core_ids=list(range(8)))
    return unshard_output(res.results)



# revision 3
# speedup vs baseline: 1.6617x; 1.6617x over previous
"""NoisyDense forward for Trainium2, 8-core tensor-parallel.

out = relu(x @ (w_mu + w_sigma * outer(eps_in, eps_out)) + b_mu + b_sigma*eps_out)

Sharding: 2-way over batch x 4-way over units (8 cores).
Per core: xT shard [4096, 2048] (batch cols, pre-transposed host-side),
w shard [4096, 1024] (unit cols), both bf16.

Key algebra: for NoisyDense init w_sigma rows are identical
(w_sigma = full(sigma)), so

  W = w_mu + w_sigma * outer(eps_in, eps_out)
    = w_mu + outer(eps_in, s_out),       s_out[n] = w_sigma[0,n] * eps_out[n]
  x @ W = x @ w_mu + outer(v, s_out),    v = x @ eps_in   (tiny matvec, host)

so the device only runs the dense GEMM x @ w_mu; the rank-1 noise term and
bias fold into the PSUM eviction as one scalar_tensor_tensor + add + relu.
No noisy-W materialization pass, no PE transposes (x is shipped transposed),
PE does nothing but the 1024 [128x128]@[128x512] bf16 matmuls per core.

For a general (non-row-constant) w_sigma the host composes the effective
W = w_mu + w_sigma*outer(eps_in,eps_out) instead and ships v=0, s_out=0;
the device program is identical.

Schedule: k-blocked over W chunks — each pair of m-panels keeps 4 PSUM
accumulation groups open and consumes W k-chunks as they stream in, so the
PE starts ~10us into the kernel instead of waiting for the full 8.4MB W.
"""

import numpy as np

BATCH = 4096
IN_DIM = 4096
UNITS = 4096
MSHARDS = 2
NSHARDS = 4
MS = BATCH // MSHARDS      # 2048 rows of x per core
NS = UNITS // NSHARDS      # 1024 units per core
P = 128
KO = IN_DIM // P           # 32 k-tiles
MP = MS // P               # 16 m-panels per core
NFREE = 512                # matmul moving free dim (one PSUM bank of fp32)
NT = NS // NFREE           # 2 n-tiles per core
WCH = 4                    # k-tiles per W dma chunk
NWCH = KO // WCH           # 8 W chunks
PAIRS = MP // 2            # 8 m-panel pairs (one x slab each)

_NC_CACHE = {}


def _build():
    from concourse import bacc
    import concourse.mybir as mybir
    import concourse.tile as tile

    f32 = mybir.dt.float32
    bf16 = mybir.dt.bfloat16

    nc = bacc.Bacc(None, target_bir_lowering=False, dynamic_dma_scratch_size=2048)

    xt_d = nc.dram_tensor("xt_s", [IN_DIM, MS], bf16, kind="ExternalInput")
    wmu_d = nc.dram_tensor("wmu_s", [IN_DIM, NS], bf16, kind="ExternalInput")
    v_d = nc.dram_tensor("v_s", [MS], f32, kind="ExternalInput")
    sout_d = nc.dram_tensor("sout_s", [NS], f32, kind="ExternalInput")
    bp_d = nc.dram_tensor("bp_s", [NS], f32, kind="ExternalInput")
    out_d = nc.dram_tensor("out_s", [MS, NS], f32, kind="ExternalOutput")

    mult = mybir.AluOpType.mult
    add = mybir.AluOpType.add

    with tile.TileContext(nc) as tc:
        with (
            tc.tile_pool(name="const", bufs=1) as const,
            tc.tile_pool(name="wpool", bufs=1) as wpool,
            tc.tile_pool(name="xsl", bufs=2) as xsl,
            tc.tile_pool(name="outp", bufs=2) as outp,
            tc.tile_pool(name="ps", bufs=8, space="PSUM") as psp,
        ):
            xt_r = xt_d[:].rearrange("(ko ki) m -> ki ko m", ki=P)
            wmu_r = wmu_d[:].rearrange("(ko ki) n -> ki ko n", ki=P)

            # ---- x slab for pair 0 first: PE's first matmul needs it ----
            def issue_x(pair):
                t = xsl.tile([P, KO, 2 * P], bf16, tag="x")
                with nc.allow_non_contiguous_dma(reason="512B k-tile rows"):
                    nc.sync.dma_start(
                        t[:], xt_r[:, :, pair * 2 * P : (pair + 1) * 2 * P]
                    )
                return t

            xslabs = {0: issue_x(0)}

            # ---- W chunks stream in; matmuls consume them chunk-by-chunk ----
            wch = []
            for c in range(NWCH):
                wt = wpool.tile([P, WCH, NS], bf16, tag=f"w{c}")
                nc.sync.dma_start(wt[:], wmu_r[:, c * WCH : (c + 1) * WCH, :])
                wch.append(wt)

            # ---- constants (small, off the critical DMA path) ----
            vcol = const.tile([P, MP], f32, tag="vcol")
            with nc.allow_non_contiguous_dma(reason="one-time 8KB strided load"):
                nc.sync.dma_start(
                    vcol[:], v_d[:].rearrange("(mp mi) -> mi mp", mi=P)
                )
            sout_b = const.tile([P, NS], f32, tag="soutb")
            bp_b = const.tile([P, NS], f32, tag="bpb")
            with nc.allow_non_contiguous_dma(reason="one-time row broadcasts"):
                nc.sync.dma_start(sout_b[:], sout_d[None, :].to_broadcast([P, NS]))
                nc.sync.dma_start(bp_b[:], bp_d[None, :].to_broadcast([P, NS]))

            xslabs[1] = issue_x(1)

            for pair in range(PAIRS):
                xs = xslabs.pop(pair)
                if pair + 2 < PAIRS:
                    xslabs[pair + 2] = issue_x(pair + 2)

                # 4 open PSUM groups: (panel-in-pair, n-tile)
                pst = [
                    psp.tile([P, NFREE], f32, tag="ps", name=f"ps{pair}_{t}")
                    for t in range(4)
                ]
                for c in range(NWCH):
                    for t in range(4):
                        pp, nt = divmod(t, 2)
                        for j in range(WCH):
                            ko = c * WCH + j
                            nc.tensor.matmul(
                                pst[t][:],
                                xs[:, ko, pp * P : (pp + 1) * P],
                                wch[c][:, j, nt * NFREE : (nt + 1) * NFREE],
                                start=(ko == 0),
                                stop=(ko == KO - 1),
                            )

                for pp in range(2):
                    pm = pair * 2 + pp
                    ot = outp.tile([P, NS], f32, tag="ot")
                    for nt in range(NT):
                        nsl = slice(nt * NFREE, (nt + 1) * NFREE)
                        # ot = s_out * v + psum  (rank-1 noise term)
                        nc.vector.scalar_tensor_tensor(
                            out=ot[:, nsl],
                            in0=sout_b[:, nsl],
                            scalar=vcol[:, pm : pm + 1],
                            in1=pst[pp * 2 + nt][:],
                            op0=mult,
                            op1=add,
                        )
                    # + bias, relu
                    nc.vector.tensor_add(ot[:], ot[:], bp_b[:])
                    nc.vector.tensor_scalar_max(ot[:], ot[:], 0.0)
                    nc.sync.dma_start(out_d[pm * P : (pm + 1) * P, :], ot[:])

    nc.compile()
    return nc


def get_nc(variant="rank1", mm_dtype_name=None):
    if "nc" not in _NC_CACHE:
        _NC_CACHE["nc"] = _build()
    return _NC_CACHE["nc"]


def pick_variant(w_sigma):
    w_sigma = np.asarray(w_sigma)
    return "rowsig" if bool((w_sigma == w_sigma[0:1, :]).all()) else "general"


def shard_inputs(x, w_mu, w_sigma, b_mu, b_sigma, eps_in, eps_out, variant=None):
    import ml_dtypes

    bf16 = ml_dtypes.bfloat16
    f32 = np.float32
    x = np.asarray(x, dtype=f32)
    w_mu = np.asarray(w_mu, dtype=f32)
    w_sigma = np.asarray(w_sigma, dtype=f32)
    b_mu = np.asarray(b_mu, dtype=f32)
    b_sigma = np.asarray(b_sigma, dtype=f32)
    eps_in = np.asarray(eps_in, dtype=f32)
    eps_out = np.asarray(eps_out, dtype=f32)

    if variant is None:
        variant = pick_variant(w_sigma)

    if variant == "rowsig":
        w_dev = w_mu.astype(bf16)
        sout = (w_sigma[0] * eps_out).astype(f32)
        v = (x @ eps_in).astype(f32)
    else:
        w_eff = w_mu + w_sigma * np.outer(eps_in, eps_out)
        w_dev = w_eff.astype(bf16)
        sout = np.zeros(UNITS, f32)
        v = np.zeros(BATCH, f32)
    bp = (b_mu + b_sigma * eps_out).astype(f32)
    xT = np.ascontiguousarray(x.astype(bf16).T)  # [IN_DIM, BATCH]

    in_maps = []
    for c in range(MSHARDS * NSHARDS):
        mr, ncol = divmod(c, NSHARDS)
        msl = slice(mr * MS, (mr + 1) * MS)
        nsl = slice(ncol * NS, (ncol + 1) * NS)
        m = {
            "xt_s": np.ascontiguousarray(xT[:, msl]),
            "wmu_s": np.ascontiguousarray(w_dev[:, nsl]),
            "v_s": np.ascontiguousarray(v[msl]),
            "sout_s": np.ascontiguousarray(sout[nsl]),
            "bp_s": np.ascontiguousarray(bp[nsl]),
        }
        in_maps.append(m)
    return in_maps


def unshard_output(results):
    out = np.empty((BATCH, UNITS), dtype=np.float32)
    for c, rmap in enumerate(results):
        mr, ncol = divmod(c, NSHARDS)
        out[mr * MS : (mr + 1) * MS, ncol * NS : (ncol + 1) * NS] = rmap["out_s"]
    return out


def kernel(x, w_mu, w_sigma, b_mu, b_sigma, eps_in, eps_out):
    from concourse.bass_utils import run_bass_kernel_spmd

    nc = get_nc()
    in_maps = shard_inputs(x, w_mu, w_sigma, b_mu, b_sigma, eps_in, eps_out)
    res = run_bass_kernel_spmd(nc, in_maps, core_ids=list(range(8)))
    return unshard_output(res.results)
